# revision 1
# baseline (speedup 1.0000x reference)
"""Trainium2 Bass kernel for nn_GroupAssignment.

Shapes (hardcoded): v_rel (1, 256, 128, 256) f32, HID=32, N=256, T=128.

Outputs:
  v_grouped  (1, 256, 128, 256) f32 -- straight-through:
             (v - v_soft) + v_soft == v exactly up to rounding, so this is
             the input passed through.
  group_idx  (1, 256) int32 -- serial union-find over dm (row-major
             lower-triangular close pairs), then rank-relabel.
  dist_full  (1, 128, 256, 256) f32 -- dm broadcast over T.

The only real tensor computation is the 256x256 distance matrix

    dm[n,m] = 0.5*(E[n,m] + E[m,n]),
    E[n,m]  = exp(c0 + sum_o a_o * relu(y[o,n] - y[o,m] + b1[o]))

with y = w1 @ v_rel[0,:2,-1,:] (32, 256), a = w2*gamma/sqrt(var+eps) and
c0 = b2 + sum_o w2_o*(beta - mean*gamma/sqrt(var+eps))_o  (BN folded).

Device strategy (8 NeuronCores, column-sharded): core i computes
dm[:, 32i:32i+32] as exp'(P[n,m]) + exp'(P[m,n]) via two passes of a
single k=33 PE matmul trick (no transposes, no collectives):

    out[p=n, f=(m,o)] = sum_k lhsT[k,p]*rhs[k,f]
    lhsT = [y; 1] (pass1) or [-y; 1] (pass2), rhs rows 0..31 carry
    a_o-scaled block-diagonal indicators and row 32 carries
    a_o*(b1_o -/+ y[o,m]).  This yields a_o*(y_on - y_om + b1_o) for all
    (m, o) in one shot.  Channels are pre-sorted so a>=0 comes first:
    relu(a*t) = a*relu(t) for a>0 via ACT Relu, and a*relu(t) = min(a*t, 0)
    for a<0 via DVE min.  A single X-reduce over o then ACT Exp (with the
    0.5 symmetrization factor folded into the bias as ln(0.5)) finishes E'.
"""

import numpy as np

TH = 1.0
EPS_BN = 1e-5

_CACHE = {}


def _build_program(npos):
    from contextlib import ExitStack

    import concourse.tile as tile
    from concourse import bacc, mybir

    FP = mybir.dt.float32
    nc = bacc.Bacc("TRN2", target_bir_lowering=False, debug=False, num_devices=8)

    lhs = nc.dram_tensor("lhs", [33, 512], FP, kind="ExternalInput").ap()
    rhs = nc.dram_tensor("rhs", [33, 2048], FP, kind="ExternalInput").ap()
    cbias = nc.dram_tensor("cbias", [128, 1], FP, kind="ExternalInput").ap()
    dmcol = nc.dram_tensor("dmcol", [256, 32], FP, kind="ExternalOutput").ap()

    with tile.TileContext(nc, trace_sim=False) as tc:
        with ExitStack() as ctx:
            consts = ctx.enter_context(tc.tile_pool(name="consts", bufs=1))
            pool = ctx.enter_context(tc.tile_pool(name="work", bufs=2))
            ppool = ctx.enter_context(tc.tile_pool(name="psum", bufs=2, space="PSUM"))

            lhs_t = consts.tile([33, 512], FP)
            nc.sync.dma_start(lhs_t[:], lhs)
            rhs_t = consts.tile([33, 2048], FP)
            nc.sync.dma_start(rhs_t[:], rhs)
            cb_t = consts.tile([128, 1], FP)
            nc.sync.dma_start(cb_t[:], cbias)

            for h in (0, 1):  # row halves n in [128h, 128h+128)
                ex = []
                for p in (0, 1):  # pass 0: E'[n, mslice]; pass 1: E'[mslice, n]
                    ps = ppool.tile([128, 1024], FP, tag="ps")
                    for mm in (0, 1):
                        nc.tensor.matmul(
                            ps[:, mm * 512 : (mm + 1) * 512],
                            lhs_t[:, p * 256 + h * 128 : p * 256 + h * 128 + 128],
                            rhs_t[:, p * 1024 + mm * 512 : p * 1024 + (mm + 1) * 512],
                            start=True,
                            stop=True,
                        )
                    hs = pool.tile([128, 1024], FP, tag="hs")
                    pv = ps[:].rearrange("p (m o) -> p m o", o=32)
                    hv = hs[:].rearrange("p (m o) -> p m o", o=32)
                    if npos > 0:
                        nc.scalar.activation(
                            hv[:, :, 0:npos],
                            pv[:, :, 0:npos],
                            mybir.ActivationFunctionType.Relu,
                        )
                    if npos < 32:
                        nc.vector.tensor_scalar_min(
                            hv[:, :, npos:32], pv[:, :, npos:32], 0.0
                        )
                    red = pool.tile([128, 32], FP, tag="red")
                    nc.vector.tensor_reduce(
                        red[:],
                        hv,
                        axis=mybir.AxisListType.X,
                        op=mybir.AluOpType.add,
                    )
                    e = pool.tile([128, 32], FP, tag=f"ex{p}")
                    nc.scalar.activation(
                        e[:],
                        red[:],
                        mybir.ActivationFunctionType.Exp,
                        bias=cb_t[:],
                        scale=1.0,
                    )
                    ex.append(e)
                s = pool.tile([128, 32], FP, tag="s")
                nc.vector.tensor_add(s[:], ex[0][:], ex[1][:])
                nc.sync.dma_start(dmcol[h * 128 : (h + 1) * 128, :], s[:])

    nc.compile()
    return nc


def _scan_groups(dist, th=TH):
    """Exact equivalent of the reference's 65536-step serial union-find scan.

    Within a row r only comp(r)'s label changes, and every genuinely-merging
    edge (r,c) relabels the merged set to comp(c)'s row-start label, so the
    final label is that of the last first-occurring distinct component among
    ok columns; the merged set is comp(r) plus all ok columns' components.
    """
    N = dist.shape[0]
    labels = np.arange(N)
    for r in range(1, N):
        ok = dist[r, :r] <= th
        if not ok.any():
            continue
        lr = labels[r]
        okl = labels[:r][ok]
        cand = okl[okl != lr]
        if cand.size:
            _, first_idx = np.unique(cand, return_index=True)
            new_label = cand[first_idx.max()]
            member = np.isin(labels, cand) | (labels == lr)
            labels = np.where(member, new_label, labels)
    present = np.zeros(N, np.int64)
    present[labels] = 1
    ranks = np.cumsum(present) - 1
    return ranks[labels].astype(np.int32)


def kernel(**inputs):
    from concourse.bass_utils import run_bass_kernel_spmd

    v_rel = np.asarray(inputs["v_rel"], dtype=np.float32)
    w1 = np.asarray(inputs["w1"], dtype=np.float32)
    b1 = np.asarray(inputs["b1"], dtype=np.float32)
    gamma = np.asarray(inputs["bn_gamma"], dtype=np.float32)
    beta = np.asarray(inputs["bn_beta"], dtype=np.float32)
    mean = np.asarray(inputs["bn_mean"], dtype=np.float32)
    var = np.asarray(inputs["bn_var"], dtype=np.float32)
    w2 = np.asarray(inputs["w2"], dtype=np.float32)
    b2 = np.asarray(inputs["b2"], dtype=np.float32)

    B, C, T, N = v_rel.shape
    assert (B, C, T, N) == (1, 256, 128, 256)

    # ---- parameter folding (host; O(N*HID) flops) ----
    x = v_rel[0, :2, -1, :]                       # (2, N)
    y = (w1 @ x).astype(np.float32)               # (32, N)
    sc = (gamma / np.sqrt(var + EPS_BN)).astype(np.float32)
    a = (w2[0] * sc).astype(np.float32)           # (32,)
    c0 = np.float32(b2[0] + np.sum(w2[0] * (beta - mean * sc)))
    c0p = np.float32(c0 + np.float32(np.log(0.5)))  # fold the 0.5 symmetrize

    perm = np.argsort((a < 0).astype(np.int8), kind="stable")  # a>=0 first
    ap_, b1p, yp = a[perm], b1[perm], y[perm]
    npos = int((ap_ >= 0).sum())

    key = ("v1", npos)
    if key not in _CACHE:
        _CACHE[key] = _build_program(npos)
    nc = _CACHE[key]

    # ---- per-core inputs ----
    lhs = np.empty((33, 512), np.float32)
    lhs[:32, :256] = yp
    lhs[:32, 256:] = -yp
    lhs[32, :] = 1.0

    base = np.zeros((33, 32, 32), np.float32)     # [k, m, o]
    for o in range(32):
        base[o, :, o] = ap_[o]
    ab = (ap_ * b1p)[None, :]                      # (1, 32o)
    cb_arr = np.full((128, 1), c0p, np.float32)

    in_maps = []
    for i in range(8):
        ms = yp[:, 32 * i : 32 * (i + 1)]          # (32o, 32m)
        r1 = base.copy()
        r1[32] = (ab - ap_[None, :] * ms.T)        # (32m, 32o)
        r2 = base.copy()
        r2[32] = (ab + ap_[None, :] * ms.T)
        rhs = np.concatenate(
            [r1.reshape(33, 1024), r2.reshape(33, 1024)], axis=1
        ).astype(np.float32)
        in_maps.append({"lhs": lhs, "rhs": np.ascontiguousarray(rhs), "cbias": cb_arr})

    res = run_bass_kernel_spmd(nc, in_maps, list(range(8)))
    dm = np.concatenate([res.results[i]["dmcol"] for i in range(8)], axis=1)
    dm = np.ascontiguousarray(dm, dtype=np.float32)             # (256, 256)

    group_idx = _scan_groups(dm)[None]                           # (1, N) int32
    v_grouped = np.array(v_rel, dtype=np.float32, copy=True)
    dist_full = np.ascontiguousarray(
        np.broadcast_to(dm[None, None], (1, T, N, N)), dtype=np.float32
    )
    return v_grouped, group_idx, dist_full


# revision 6
# speedup vs baseline: 1.3479x; 1.3479x over previous
"""Trainium2 Bass kernel for nn_GroupAssignment.

Shapes (hardcoded): v_rel (1, 256, 128, 256) f32, HID=32, N=256, T=128.

Outputs:
  v_grouped  (1, 256, 128, 256) f32 -- straight-through:
             (v - v_soft) + v_soft == v exactly up to rounding, so this is
             the input passed through.
  group_idx  (1, 256) int32 -- serial union-find over dm (row-major
             lower-triangular close pairs), then rank-relabel.
  dist_full  (1, 128, 256, 256) f32 -- dm broadcast over T.

The only real tensor computation is the 256x256 distance matrix

    dm[n,m] = 0.5*(E[n,m] + E[m,n]),
    E[n,m]  = exp(c0 + sum_o a_o * relu(y[o,n] - y[o,m] + b1[o]))

with y = w1 @ v_rel[0,:2,-1,:] (32, 256), a = w2*gamma/sqrt(var+eps) and
c0 = b2 + sum_o w2_o*(beta - mean*gamma/sqrt(var+eps))_o  (BN folded).

Device strategy (8 NeuronCores, column-sharded): core i computes
dm[:, 32i:32i+32] as exp'(P[n,m]) + exp'(P[m,n]) via two passes of a
single k=33 PE matmul trick (no transposes, no collectives):

    out[p=n, f=(m,o)] = sum_k lhsT[k,p]*rhs[k,f]
    lhsT = [y; 1] (pass1) or [-y; 1] (pass2), rhs rows 0..31 carry
    a_o-scaled block-diagonal indicators and row 32 carries
    a_o*(b1_o -/+ y[o,m]).  This yields a_o*(y_on - y_om + b1_o) for all
    (m, o) in one shot.  Channels are pre-sorted so a>=0 comes first:
    relu(a*t) = a*relu(t) for a>0 via ACT Relu, and a*relu(t) = min(a*t, 0)
    for a<0 via DVE min.  A single X-reduce over o then ACT Exp (with the
    0.5 symmetrization factor folded into the bias as ln(0.5)) finishes E'.
"""

import numpy as np

TH = 1.0
EPS_BN = 1e-5

_CACHE = {}


def _build_program(npos):
    from contextlib import ExitStack

    import concourse.tile as tile
    from concourse import bacc, mybir

    FP = mybir.dt.float32
    FR = mybir.dt.float32r  # fp32 "replay" matmul: 4x PE rate, fp32 accuracy
    nc = bacc.Bacc("TRN2", target_bir_lowering=False, debug=False, num_devices=8)

    lhs = nc.dram_tensor("lhs", [33, 512], FR, kind="ExternalInput").ap()
    rhs = nc.dram_tensor("rhs", [33, 2048], FR, kind="ExternalInput").ap()
    cbias = nc.dram_tensor("cbias", [128, 1], FP, kind="ExternalInput").ap()
    dmcol = nc.dram_tensor("dmcol", [256, 32], FP, kind="ExternalOutput").ap()

    with tile.TileContext(nc, trace_sim=False) as tc:
        with ExitStack() as ctx:
            consts = ctx.enter_context(tc.tile_pool(name="consts", bufs=1))
            pool = ctx.enter_context(tc.tile_pool(name="work", bufs=2))
            ppool = ctx.enter_context(tc.tile_pool(name="psum", bufs=2, space="PSUM"))

            lhs_t = consts.tile([33, 512], FR)
            nc.sync.dma_start(lhs_t[:], lhs)
            rhs_t = consts.tile([33, 2048], FR)
            nc.sync.dma_start(rhs_t[:], rhs)
            cb_t = consts.tile([128, 1], FP)
            nc.sync.dma_start(cb_t[:], cbias)

            for h in (0, 1):  # row halves n in [128h, 128h+128)
                ex = []
                for p in (0, 1):  # pass 0: E'[n, mslice]; pass 1: E'[mslice, n]
                    ps = ppool.tile([128, 1024], FP, tag="ps")
                    for mm in (0, 1):
                        nc.tensor.matmul(
                            ps[:, mm * 512 : (mm + 1) * 512],
                            lhs_t[:, p * 256 + h * 128 : p * 256 + h * 128 + 128],
                            rhs_t[:, p * 1024 + mm * 512 : p * 1024 + (mm + 1) * 512],
                            start=True,
                            stop=True,
                        )
                    hs = pool.tile([128, 1024], FP, tag="hs")
                    pv = ps[:].rearrange("p (m o) -> p m o", o=32)
                    hv = hs[:].rearrange("p (m o) -> p m o", o=32)
                    if npos > 0:
                        nc.scalar.activation(
                            hv[:, :, 0:npos],
                            pv[:, :, 0:npos],
                            mybir.ActivationFunctionType.Relu,
                        )
                    if npos < 32:
                        nc.vector.tensor_scalar_min(
                            hv[:, :, npos:32], pv[:, :, npos:32], 0.0
                        )
                    red = pool.tile([128, 32], FP, tag="red")
                    nc.vector.tensor_reduce(
                        red[:],
                        hv,
                        axis=mybir.AxisListType.X,
                        op=mybir.AluOpType.add,
                    )
                    e = pool.tile([128, 32], FP, tag=f"ex{p}")
                    nc.scalar.activation(
                        e[:],
                        red[:],
                        mybir.ActivationFunctionType.Exp,
                        bias=cb_t[:],
                        scale=1.0,
                    )
                    ex.append(e)
                s = pool.tile([128, 32], FP, tag="s")
                nc.vector.tensor_add(s[:], ex[0][:], ex[1][:])
                nc.sync.dma_start(dmcol[h * 128 : (h + 1) * 128, :], s[:])

    nc.compile()
    return nc


def _scan_groups(dist, th=TH):
    """Exact equivalent of the reference's 65536-step serial union-find scan.

    Within a row r only comp(r)'s label changes, and every genuinely-merging
    edge (r,c) relabels the merged set to comp(c)'s row-start label, so the
    final label is that of the last first-occurring distinct component among
    ok columns; the merged set is comp(r) plus all ok columns' components.
    """
    N = dist.shape[0]
    labels = np.arange(N)
    for r in range(1, N):
        ok = dist[r, :r] <= th
        if not ok.any():
            continue
        lr = labels[r]
        okl = labels[:r][ok]
        cand = okl[okl != lr]
        if cand.size:
            _, first_idx = np.unique(cand, return_index=True)
            new_label = cand[first_idx.max()]
            member = np.isin(labels, cand) | (labels == lr)
            labels = np.where(member, new_label, labels)
    present = np.zeros(N, np.int64)
    present[labels] = 1
    ranks = np.cumsum(present) - 1
    return ranks[labels].astype(np.int32)


def kernel(**inputs):
    from concourse.bass_utils import run_bass_kernel_spmd

    v_rel = np.asarray(inputs["v_rel"], dtype=np.float32)
    w1 = np.asarray(inputs["w1"], dtype=np.float32)
    b1 = np.asarray(inputs["b1"], dtype=np.float32)
    gamma = np.asarray(inputs["bn_gamma"], dtype=np.float32)
    beta = np.asarray(inputs["bn_beta"], dtype=np.float32)
    mean = np.asarray(inputs["bn_mean"], dtype=np.float32)
    var = np.asarray(inputs["bn_var"], dtype=np.float32)
    w2 = np.asarray(inputs["w2"], dtype=np.float32)
    b2 = np.asarray(inputs["b2"], dtype=np.float32)

    B, C, T, N = v_rel.shape
    assert (B, C, T, N) == (1, 256, 128, 256)

    # ---- parameter folding (host; O(N*HID) flops) ----
    x = v_rel[0, :2, -1, :]                       # (2, N)
    y = (w1 @ x).astype(np.float32)               # (32, N)
    sc = (gamma / np.sqrt(var + EPS_BN)).astype(np.float32)
    a = (w2[0] * sc).astype(np.float32)           # (32,)
    c0 = np.float32(b2[0] + np.sum(w2[0] * (beta - mean * sc)))
    c0p = np.float32(c0 + np.float32(np.log(0.5)))  # fold the 0.5 symmetrize

    perm = np.argsort((a < 0).astype(np.int8), kind="stable")  # a>=0 first
    ap_, b1p, yp = a[perm], b1[perm], y[perm]
    npos = int((ap_ >= 0).sum())

    key = ("v1", npos)
    if key not in _CACHE:
        _CACHE[key] = _build_program(npos)
    nc = _CACHE[key]

    # ---- per-core inputs ----
    lhs = np.empty((33, 512), np.float32)
    lhs[:32, :256] = yp
    lhs[:32, 256:] = -yp
    lhs[32, :] = 1.0

    base = np.zeros((33, 32, 32), np.float32)     # [k, m, o]
    for o in range(32):
        base[o, :, o] = ap_[o]
    ab = (ap_ * b1p)[None, :]                      # (1, 32o)
    cb_arr = np.full((128, 1), c0p, np.float32)

    in_maps = []
    for i in range(8):
        ms = yp[:, 32 * i : 32 * (i + 1)]          # (32o, 32m)
        r1 = base.copy()
        r1[32] = (ab - ap_[None, :] * ms.T)        # (32m, 32o)
        r2 = base.copy()
        r2[32] = (ab + ap_[None, :] * ms.T)
        rhs = np.concatenate(
            [r1.reshape(33, 1024), r2.reshape(33, 1024)], axis=1
        ).astype(np.float32)
        in_maps.append({"lhs": lhs, "rhs": np.ascontiguousarray(rhs), "cbias": cb_arr})

    res = run_bass_kernel_spmd(nc, in_maps, list(range(8)))
    dm = np.concatenate([res.results[i]["dmcol"] for i in range(8)], axis=1)
    dm = np.ascontiguousarray(dm, dtype=np.float32)             # (256, 256)

    group_idx = _scan_groups(dm)[None]                           # (1, N) int32
    v_grouped = np.array(v_rel, dtype=np.float32, copy=True)
    dist_full = np.ascontiguousarray(
        np.broadcast_to(dm[None, None], (1, T, N, N)), dtype=np.float32
    )
    return v_grouped, group_idx, dist_full
